# revision 37
# baseline (speedup 1.0000x reference)
"""SLAYER SNN forward kernel for Trainium2 (8 NeuronCores, data-parallel over batch).

Network (per reference): x:[B,2048,350] -> psp(srm) -> W1 -> spike-scan ->
psp(srm) -> W2 -> spike-scan -> s2:[B,10,350].

Math restructuring (exact up to fp32 reassociation):
  - psp is a causal linear filter along t; it commutes with the dense layer:
      a1 = einsum(psp(x), W1) == psp(einsum(x, W1))
    so the big matmul runs on the raw binary spikes (exact in bf16) and the
    100-tap filter runs on the small [512, T] result as a banded-Toeplitz
    matmul against a constant K matrix.
  - the refractory kernel refk[j] = -20 * j * e^(1-j) is the impulse response
    of a 2nd-order linear recurrence (double pole at rho=e^-1).  With scaled
    states P,Q (P = refractory potential / (-20), dividing by -20 flips the
    threshold comparison):
        s[t] = (P[t] <= vhat[t]),   vhat = (u - theta)/20
        Q <- rho*Q + s[t]
        P <- rho*P + Q
    The scan is emitted as 4 fused VectorE ops per step in a depth-2 schedule
    (W = vhat - rho*P precomputed) so only two ops per step sit on the
    semaphore-latency-bound dependency chain; all 2048 neuron-batch units per
    core advance together ([128, 16] per op, t-major buffers so per-step
    slices are contiguous).
  - layer 2 never comes near threshold (|a2| < 1 << theta=10), so its scan is
    computed by fixpoint iteration: bulk IIR scans along t (tensor_tensor_scan)
    + threshold, iterated K2=3 times — exact unless a 3-long chained
    refractory flip-cascade exists, impossible with a 9+ margin to theta.

Sharding: batch 32 -> 8 cores x 4.  W1/W2/K replicated.
"""

import numpy as np
import ml_dtypes

B_FULL = 32
N_CORES = 8
B_LOC = B_FULL // N_CORES  # 4
NIN = 2048
NHID = 512
NOUT = 10
T = 350
THETA = 10.0
K_SRM = 100

NC_IN = NIN // 128  # 16 contraction chunks
MT_N = NHID // 128  # 4 hidden m-tiles
G = B_LOC * MT_N    # 16 scan groups per core
TCH = [(0, 128), (128, 128), (256, 94)]  # (offset, size) t' chunks
RHO = float(np.float32(np.exp(np.float64(-1.0))))
CRHO = -20.0  # refk[1]; refk[j] = CRHO * j * RHO**(j-1)
VSCALE = 0.05         # 1/|CRHO|, exact in fp32
VBIAS = -0.5          # -THETA/|CRHO|, exact
# spike condition: m = u + CRHO*P >= theta  <=>  P <= (u-theta)/20 = vhat
K2_FIX = 3

bf16 = ml_dtypes.bfloat16


def _srm_np():
    t = np.arange(K_SRM, dtype=np.float32)
    return ((t / np.float32(10.0)) * np.exp(np.float32(1.0) - t / np.float32(10.0))).astype(np.float32)


def _kmat_np():
    """K[c, p, t] = srm[t - (128c + p)], zero outside [0, K_SRM)."""
    srm = _srm_np()
    k = np.zeros((3, 128, T), dtype=np.float32)
    for c in range(3):
        for p in range(TCH[c][1]):
            tp = 128 * c + p
            j0, j1 = tp, min(T, tp + K_SRM)
            k[c, p, j0:j1] = srm[: j1 - j0]
    return k


def build_program(debug_taps: bool = False):
    import concourse.bass as bass
    import concourse.tile as tile
    from concourse import bacc, mybir

    f32 = mybir.dt.float32
    bfl = mybir.dt.bfloat16
    OP = mybir.AluOpType
    ACTF = mybir.ActivationFunctionType

    nc = bacc.Bacc("TRN2", target_bir_lowering=False, debug=False,
                   enable_asserts=False, num_devices=N_CORES)

    x_d = nc.dram_tensor("x", [B_LOC, NIN, T], bfl, kind="ExternalInput").ap()
    w1t_d = nc.dram_tensor("w1t", [NIN, NHID], bfl, kind="ExternalInput").ap()
    w2t_d = nc.dram_tensor("w2t", [NHID, NOUT], bfl, kind="ExternalInput").ap()
    out_d = nc.dram_tensor("out", [B_LOC, NOUT, T], f32, kind="ExternalOutput").ap()
    kmat_d = nc.inline_tensor(_kmat_np().astype(bf16), name="kmat").ap()
    if debug_taps:
        dbg_v = nc.dram_tensor("dbg_v", [128, T, G], f32, kind="ExternalOutput").ap()
        dbg_s = nc.dram_tensor("dbg_s", [128, T, G], mybir.dt.bfloat16,
                               kind="ExternalOutput").ap()
        dbg_v2 = nc.dram_tensor("dbg_v2", [B_LOC * NOUT, T], f32,
                                kind="ExternalOutput").ap()

    with tile.TileContext(nc) as tc:
        with (
            tc.tile_pool(name="singles", bufs=1) as singles,
            tc.tile_pool(name="xin", bufs=1) as xin,
            tc.tile_pool(name="z1sb", bufs=1) as z1sb,
            tc.tile_pool(name="scan", bufs=1) as scan,
            tc.tile_pool(name="l2", bufs=1) as l2p,
            tc.tile_pool(name="zps", bufs=4, space="PSUM") as zps,
            tc.tile_pool(name="mmps", bufs=3, space="PSUM") as mmps,
        ):
            # ---- constants ----
            w1t_sb = singles.tile([128, NC_IN, NHID], bfl)
            for c4 in range(4):
                nc.sync.dma_start(
                    out=w1t_sb[:, c4 * 4:(c4 + 1) * 4, :],
                    in_=w1t_d[c4 * 512:(c4 + 1) * 512].rearrange(
                        "(c p) m -> p c m", p=128))
            w2t_sb = singles.tile([128, MT_N, NOUT], bfl)
            nc.gpsimd.dma_start(out=w2t_sb, in_=w2t_d.rearrange("(c p) o -> p c o", p=128))
            kmat_sb = singles.tile([128, 3, T], bfl)
            for c in range(3):
                nc.gpsimd.dma_start(out=kmat_sb[:, c, :], in_=kmat_d[c])
            rho_sb = singles.tile([128, T], f32)
            nc.vector.memset(rho_sb, RHO)

            # ---- persistent scan buffers (t-major: per-step slices contiguous) ----
            v_all = scan.tile([128, T, G], f32)       # vhat for all 16 groups
            s_all = scan.tile([128, T + 2, G], bfl)   # t=0 slice = zero guard
            a_st = scan.tile([128, G], f32)           # P state
            b_st = scan.tile([128, G], f32)           # Q state
            nc.vector.memset(s_all[:, 0, :], 0.0)
            nc.vector.memset(a_st, 0.0)
            nc.vector.memset(b_st, 0.0)

            # ---- layer 1, t-chunk-major so the scan can start after chunk 0:
            # for each t'-chunk: z1T chunk for all batches, then a1/vhat for
            # the t-columns this chunk completes.
            x_tiles = []
            dma_engines = [nc.gpsimd, nc.sync, nc.gpsimd, nc.sync]
            for b in range(B_LOC):
                x_sb = xin.tile([128, NC_IN, T], bfl, tag=f"x{b}", name=f"x_sb{b}")
                for c4 in range(4):
                    dma_engines[b].dma_start(
                        out=x_sb[:, c4 * 4:(c4 + 1) * 4, :],
                        in_=x_d[b][c4 * 512:(c4 + 1) * 512].rearrange(
                            "(c p) t -> p c t", p=128))
                x_tiles.append(x_sb)
            z1_tiles = [z1sb.tile([128, 3, NHID], bfl, tag=f"z1{b}", name=f"z1t{b}")
                        for b in range(B_LOC)]

            for b in range(B_LOC):
                for tc_i, (toff, tsz) in enumerate(TCH):
                    z1ps = zps.tile([128, NHID], f32, tag="zps")
                    for ncnk in range(NC_IN):
                        nc.tensor.matmul(
                            z1ps[:tsz, :],
                            x_tiles[b][:, ncnk, toff:toff + tsz],
                            w1t_sb[:, ncnk, :],
                            start=(ncnk == 0), stop=(ncnk == NC_IN - 1),
                        )
                    nc.scalar.activation(out=z1_tiles[b][:tsz, tc_i, :],
                                         in_=z1ps[:tsz, :], func=ACTF.Copy)
                for mt in range(MT_N):
                    g = b * MT_N + mt
                    a1ps = mmps.tile([128, T], f32, tag="mmps")
                    for cj, (tj, szj) in enumerate(TCH):
                        cis = [cj] if cj == 0 else [cj - 1, cj]
                        for idx, ci in enumerate(cis):
                            szi = TCH[ci][1]
                            nc.tensor.matmul(
                                a1ps[:, tj:tj + szj],
                                z1_tiles[b][:szi, ci, mt * 128:(mt + 1) * 128],
                                kmat_sb[:szi, ci, tj:tj + szj],
                                start=(idx == 0), stop=(idx == len(cis) - 1),
                            )
                    # vhat = (a1 - theta)/CRHO = a1*VSCALE + VBIAS
                    nc.scalar.activation(out=v_all[:, :, g], in_=a1ps,
                                         func=ACTF.Copy, scale=VSCALE, bias=VBIAS)

            # ---- layer 1 spike scan ----
            # Depth-2 form: W_t = vhat_t - rho*A_{t-1}; s_t = (B_t <= W_t);
            # A_t = rho*A_{t-1} + B_t; B_{t+1} = rho*B_t + s_t.
            # Per-step critical chain is only 2 ops (s_t<-W_t, B<-s_t); the
            # other two ops' inputs are >=2 ops old, hiding the SBUF
            # read-after-write bubble that dominates small VectorE ops.
            w_tmp = scan.tile([128, G], f32)
            for t in range(T):
                nc.vector.scalar_tensor_tensor(w_tmp, a_st, -RHO, v_all[:, t, :],
                                               OP.mult, OP.add)
                nc.vector.scalar_tensor_tensor(a_st, a_st, RHO, b_st, OP.mult, OP.add)
                nc.vector.tensor_tensor(s_all[:, t + 1, :], b_st, w_tmp, OP.is_le)
                nc.vector.scalar_tensor_tensor(b_st, b_st, RHO, s_all[:, t + 1, :],
                                               OP.mult, OP.add)

            # ---- layer 2: z2T[t, (b,o)] = s1^T W2^T ; a2 = K^T-conv ----
            z2t_sb = l2p.tile([128, 3, B_LOC * NOUT], bfl)
            for tc_i, (toff, tsz) in enumerate(TCH):
                z2ps = mmps.tile([128, B_LOC * NOUT], f32, tag="mmps")
                for b in range(B_LOC):
                    for mt in range(MT_N):
                        g = b * MT_N + mt
                        nc.tensor.matmul(
                            z2ps[:tsz, b * NOUT:(b + 1) * NOUT],
                            s_all[:, 1 + toff:1 + toff + tsz, g],
                            w2t_sb[:, mt, :],
                            start=(mt == 0), stop=(mt == MT_N - 1),
                        )
                nc.scalar.activation(out=z2t_sb[:tsz, tc_i, :], in_=z2ps[:tsz, :],
                                     func=ACTF.Copy)

            a2ps = mmps.tile([B_LOC * NOUT, T], f32, tag="mmps")
            for cj, (tj, szj) in enumerate(TCH):
                cis = [cj] if cj == 0 else [cj - 1, cj]
                for idx, ci in enumerate(cis):
                    ti, szi = TCH[ci]
                    nc.tensor.matmul(
                        a2ps[:, tj:tj + szj],
                        z2t_sb[:szi, ci, :],
                        kmat_sb[:szi, ci, tj:tj + szj],
                        start=(idx == 0), stop=(idx == len(cis) - 1),
                    )
            v2 = l2p.tile([B_LOC * NOUT, T], f32)
            nc.scalar.activation(out=v2, in_=a2ps, func=ACTF.Copy,
                                 scale=VSCALE, bias=VBIAS)

            # ---- layer 2 spike scan via fixpoint (never near threshold) ----
            s2 = l2p.tile([B_LOC * NOUT, T + 2], bfl)
            nc.vector.memset(s2[:, 0:1], 0.0)
            nc.vector.tensor_scalar(s2[:, 1:T + 1], v2, 0.0, None, OP.is_ge)
            out_sb = l2p.tile([B_LOC * NOUT, T], f32)
            P = B_LOC * NOUT
            for it in range(K2_FIX - 1):
                x1 = l2p.tile([P, T], f32, tag="x1")
                x2 = l2p.tile([P, T], f32, tag="x2")
                nc.vector.tensor_tensor_scan(x1, rho_sb[:P, :], s2[:, 0:T], 0.0,
                                             OP.mult, OP.add)
                nc.vector.tensor_tensor_scan(x2, rho_sb[:P, :], x1, 0.0,
                                             OP.mult, OP.add)
                last = it == K2_FIX - 2
                nc.vector.tensor_tensor(out_sb if last else s2[:, 1:T + 1],
                                        x2, v2, OP.is_le)

            nc.sync.dma_start(out=out_d.rearrange("b o t -> (b o) t"), in_=out_sb)
            if debug_taps:
                nc.sync.dma_start(out=dbg_v, in_=v_all)
                nc.sync.dma_start(out=dbg_s, in_=s_all[:, 1:T + 1, :])
                nc.sync.dma_start(out=dbg_v2, in_=v2)

    nc.compile()
    return nc


def _to_bf16_binary(x):
    # spike values are exactly 0.0/1.0, which bf16 represents exactly
    return x.astype(bf16)


def kernel(spike_input: np.ndarray, W1: np.ndarray, W2: np.ndarray) -> np.ndarray:
    from concourse.bass_utils import run_bass_kernel_spmd

    nc = build_program()

    xb = _to_bf16_binary(np.ascontiguousarray(spike_input, dtype=np.float32))
    w1t = np.ascontiguousarray(W1.T).astype(bf16)
    w2t = np.ascontiguousarray(W2.T).astype(bf16)

    in_maps = []
    for c in range(N_CORES):
        in_maps.append({
            "x": np.ascontiguousarray(xb[c * B_LOC:(c + 1) * B_LOC]),
            "w1t": w1t,
            "w2t": w2t,
        })
    res = run_bass_kernel_spmd(nc, in_maps, core_ids=list(range(N_CORES)))
    out = np.concatenate([r["out"] for r in res.results], axis=0)
    return np.ascontiguousarray(out, dtype=np.float32)


def _prep_in_maps(spike_input, W1, W2):
    xb = _to_bf16_binary(np.ascontiguousarray(spike_input, dtype=np.float32))
    w1t = np.ascontiguousarray(W1.T).astype(bf16)
    w2t = np.ascontiguousarray(W2.T).astype(bf16)
    return [
        {"x": np.ascontiguousarray(xb[c * B_LOC:(c + 1) * B_LOC]),
         "w1t": w1t, "w2t": w2t}
        for c in range(N_CORES)
    ]


def _ensure_ntff_hook():
    """The RL container's antenv stub lacks axon_hooks; synthesize it and
    register the ctypes NTFF profiler from trn_agent_boot."""
    import sys
    import types
    try:
        from antenv.axon_hooks import get_axon_ntff_profile_hook  # noqa: F401
        return
    except ImportError:
        pass
    import antenv
    mod = types.ModuleType("antenv.axon_hooks")
    store = {"h": None}
    mod.set_axon_ntff_profile_hook = lambda h: store.__setitem__("h", h)
    mod.get_axon_ntff_profile_hook = lambda: store["h"]
    sys.modules["antenv.axon_hooks"] = mod
    antenv.axon_hooks = mod
    from trn_agent_boot.trn_boot import _ntff_profile_via_ctypes
    mod.set_axon_ntff_profile_hook(_ntff_profile_via_ctypes("/opt/axon/libaxon_pjrt.so"))


def profile_hw(inputs):
    """Run with NTFF tracing; return max-core exec time in ns (or None)."""
    from concourse.bass_utils import run_bass_kernel_spmd

    _ensure_ntff_hook()
    nc = build_program()
    in_maps = _prep_in_maps(**inputs)
    res = run_bass_kernel_spmd(nc, in_maps, core_ids=list(range(N_CORES)),
                               trace=True)
    return res.exec_time_ns


if __name__ == "__main__":
    x = np.zeros((B_FULL, NIN, T), np.float32)
    w1 = np.zeros((NHID, NIN), np.float32)
    w2 = np.zeros((NOUT, NHID), np.float32)
    print(kernel(x, w1, w2).shape)


# revision 39
# speedup vs baseline: 1.0384x; 1.0384x over previous
"""SLAYER SNN forward kernel for Trainium2 (8 NeuronCores, data-parallel over batch).

Network (per reference): x:[B,2048,350] -> psp(srm) -> W1 -> spike-scan ->
psp(srm) -> W2 -> spike-scan -> s2:[B,10,350].

Math restructuring (exact up to fp32 reassociation):
  - psp is a causal linear filter along t; it commutes with the dense layer:
      a1 = einsum(psp(x), W1) == psp(einsum(x, W1))
    so the big matmul runs on the raw binary spikes (exact in bf16) and the
    100-tap filter runs on the small [512, T] result as a banded-Toeplitz
    matmul against a constant K matrix.
  - the refractory kernel refk[j] = -20 * j * e^(1-j) is the impulse response
    of a 2nd-order linear recurrence (double pole at rho=e^-1).  With scaled
    states P,Q (P = refractory potential / (-20), dividing by -20 flips the
    threshold comparison):
        s[t] = (P[t] <= vhat[t]),   vhat = (u - theta)/20
        Q <- rho*Q + s[t]
        P <- rho*P + Q
    The scan is emitted as 4 fused VectorE ops per step in a depth-2 schedule
    (W = vhat - rho*P precomputed) so only two ops per step sit on the
    semaphore-latency-bound dependency chain; all 2048 neuron-batch units per
    core advance together ([128, 16] per op, t-major buffers so per-step
    slices are contiguous).
  - layer 2 never comes near threshold (|a2| < 1 << theta=10), so its scan is
    computed by fixpoint iteration: bulk IIR scans along t (tensor_tensor_scan)
    + threshold, iterated K2=3 times — exact unless a 3-long chained
    refractory flip-cascade exists, impossible with a 9+ margin to theta.

Sharding: batch 32 -> 8 cores x 4.  W1/W2/K replicated.
"""

import numpy as np
import ml_dtypes

B_FULL = 32
N_CORES = 8
B_LOC = B_FULL // N_CORES  # 4
NIN = 2048
NHID = 512
NOUT = 10
T = 350
THETA = 10.0
K_SRM = 100

NC_IN = NIN // 128  # 16 contraction chunks
MT_N = NHID // 128  # 4 hidden m-tiles
G = B_LOC * MT_N    # 16 scan groups per core
TCH = [(0, 128), (128, 128), (256, 94)]  # (offset, size) t' chunks
RHO = float(np.float32(np.exp(np.float64(-1.0))))
CRHO = -20.0  # refk[1]; refk[j] = CRHO * j * RHO**(j-1)
VSCALE = 0.05         # 1/|CRHO|, exact in fp32
VBIAS = -0.5          # -THETA/|CRHO|, exact
# spike condition: m = u + CRHO*P >= theta  <=>  P <= (u-theta)/20 = vhat
K2_FIX = 3

bf16 = ml_dtypes.bfloat16


def _srm_np():
    t = np.arange(K_SRM, dtype=np.float32)
    return ((t / np.float32(10.0)) * np.exp(np.float32(1.0) - t / np.float32(10.0))).astype(np.float32)


def _kmat_np():
    """K[c, p, t] = srm[t - (128c + p)], zero outside [0, K_SRM)."""
    srm = _srm_np()
    k = np.zeros((3, 128, T), dtype=np.float32)
    for c in range(3):
        for p in range(TCH[c][1]):
            tp = 128 * c + p
            j0, j1 = tp, min(T, tp + K_SRM)
            k[c, p, j0:j1] = srm[: j1 - j0]
    return k


def build_program(debug_taps: bool = False):
    import concourse.bass as bass
    import concourse.tile as tile
    from concourse import bacc, mybir

    f32 = mybir.dt.float32
    bfl = mybir.dt.bfloat16
    OP = mybir.AluOpType
    ACTF = mybir.ActivationFunctionType

    nc = bacc.Bacc("TRN2", target_bir_lowering=False, debug=False,
                   enable_asserts=False, num_devices=N_CORES)

    x_d = nc.dram_tensor("x", [B_LOC, NIN, T], bfl, kind="ExternalInput").ap()
    w1t_d = nc.dram_tensor("w1t", [NIN, NHID], bfl, kind="ExternalInput").ap()
    w2t_d = nc.dram_tensor("w2t", [NHID, NOUT], bfl, kind="ExternalInput").ap()
    out_d = nc.dram_tensor("out", [B_LOC, NOUT, T], f32, kind="ExternalOutput").ap()
    kmat_d = nc.inline_tensor(_kmat_np().astype(bf16), name="kmat").ap()
    if debug_taps:
        dbg_v = nc.dram_tensor("dbg_v", [128, T, G], f32, kind="ExternalOutput").ap()
        dbg_s = nc.dram_tensor("dbg_s", [128, T, G], mybir.dt.bfloat16,
                               kind="ExternalOutput").ap()
        dbg_v2 = nc.dram_tensor("dbg_v2", [B_LOC * NOUT, T], f32,
                                kind="ExternalOutput").ap()

    with tile.TileContext(nc) as tc:
        with (
            tc.tile_pool(name="singles", bufs=1) as singles,
            tc.tile_pool(name="xin", bufs=1) as xin,
            tc.tile_pool(name="z1sb", bufs=1) as z1sb,
            tc.tile_pool(name="scan", bufs=1) as scan,
            tc.tile_pool(name="l2", bufs=1) as l2p,
            tc.tile_pool(name="zps", bufs=4, space="PSUM") as zps,
            tc.tile_pool(name="mmps", bufs=3, space="PSUM") as mmps,
        ):
            # ---- constants ----
            w1t_sb = singles.tile([128, NC_IN, NHID], bfl)
            for c4 in range(4):
                nc.sync.dma_start(
                    out=w1t_sb[:, c4 * 4:(c4 + 1) * 4, :],
                    in_=w1t_d[c4 * 512:(c4 + 1) * 512].rearrange(
                        "(c p) m -> p c m", p=128))
            w2t_sb = singles.tile([128, MT_N, NOUT], bfl)
            nc.gpsimd.dma_start(out=w2t_sb, in_=w2t_d.rearrange("(c p) o -> p c o", p=128))
            kmat_sb = singles.tile([128, 3, T], bfl)
            for c in range(3):
                nc.gpsimd.dma_start(out=kmat_sb[:, c, :], in_=kmat_d[c])
            rho_sb = singles.tile([128, T], f32)
            nc.vector.memset(rho_sb, RHO)

            # ---- persistent scan buffers (t-major: per-step slices contiguous) ----
            v_all = scan.tile([128, T, G], f32)       # vhat for all 16 groups
            s_all = scan.tile([128, T + 2, G], bfl)   # t=0 slice = zero guard
            a_st = scan.tile([128, G], f32)           # P state
            b_st = scan.tile([128, G], f32)           # Q state
            nc.vector.memset(s_all[:, 0, :], 0.0)
            nc.vector.memset(a_st, 0.0)
            nc.vector.memset(b_st, 0.0)

            # ---- layer 1, t-chunk-major so the scan can start after chunk 0:
            # for each t'-chunk: z1T chunk for all batches, then a1/vhat for
            # the t-columns this chunk completes.
            x_tiles = []
            dma_engines = [nc.gpsimd, nc.sync, nc.gpsimd, nc.sync]
            for b in range(B_LOC):
                x_sb = xin.tile([128, NC_IN, T], bfl, tag=f"x{b}", name=f"x_sb{b}")
                for c4 in range(4):
                    dma_engines[b].dma_start(
                        out=x_sb[:, c4 * 4:(c4 + 1) * 4, :],
                        in_=x_d[b][c4 * 512:(c4 + 1) * 512].rearrange(
                            "(c p) t -> p c t", p=128))
                x_tiles.append(x_sb)
            z1_tiles = [z1sb.tile([128, 3, NHID], bfl, tag=f"z1{b}", name=f"z1t{b}")
                        for b in range(B_LOC)]

            # Two phases so the scan can start ~18us earlier: phase A covers
            # t'-chunks 0,1 (completing vhat cols 0..255 for every group);
            # phase B (chunk 2, cols 256..349) overlaps the scan's first steps.
            def stage_b(b, tc_i, toff, tsz):
                z1ps = zps.tile([128, NHID], f32, tag="zps", name=f"z1ps{b}_{tc_i}")
                for ncnk in range(NC_IN):
                    nc.tensor.matmul(
                        z1ps[:tsz, :],
                        x_tiles[b][:, ncnk, toff:toff + tsz],
                        w1t_sb[:, ncnk, :],
                        start=(ncnk == 0), stop=(ncnk == NC_IN - 1),
                    )
                nc.scalar.activation(out=z1_tiles[b][:tsz, tc_i, :],
                                     in_=z1ps[:tsz, :], func=ACTF.Copy)

            for b in range(B_LOC):
                for tc_i in (0, 1):
                    stage_b(b, tc_i, *TCH[tc_i])
            for b in range(B_LOC):
                for mt in range(MT_N):
                    g = b * MT_N + mt
                    a1ps = mmps.tile([128, 256], f32, tag="mmps", name=f"a1psA{g}")
                    nc.tensor.matmul(a1ps[:, 0:128],
                                     z1_tiles[b][:128, 0, mt * 128:(mt + 1) * 128],
                                     kmat_sb[:128, 0, 0:128],
                                     start=True, stop=True)
                    nc.tensor.matmul(a1ps[:, 128:256],
                                     z1_tiles[b][:128, 0, mt * 128:(mt + 1) * 128],
                                     kmat_sb[:128, 0, 128:256],
                                     start=True, stop=False)
                    nc.tensor.matmul(a1ps[:, 128:256],
                                     z1_tiles[b][:128, 1, mt * 128:(mt + 1) * 128],
                                     kmat_sb[:128, 1, 128:256],
                                     start=False, stop=True)
                    nc.scalar.activation(out=v_all[:, 0:256, g], in_=a1ps,
                                         func=ACTF.Copy, scale=VSCALE, bias=VBIAS)
            for b in range(B_LOC):
                stage_b(b, 2, *TCH[2])
            for b in range(B_LOC):
                for mt in range(MT_N):
                    g = b * MT_N + mt
                    a1ps = mmps.tile([128, 94], f32, tag="mmps", name=f"a1psB{g}")
                    nc.tensor.matmul(a1ps[:, :],
                                     z1_tiles[b][:128, 1, mt * 128:(mt + 1) * 128],
                                     kmat_sb[:128, 1, 256:350],
                                     start=True, stop=False)
                    nc.tensor.matmul(a1ps[:, :],
                                     z1_tiles[b][:94, 2, mt * 128:(mt + 1) * 128],
                                     kmat_sb[:94, 2, 256:350],
                                     start=False, stop=True)
                    nc.scalar.activation(out=v_all[:, 256:350, g], in_=a1ps,
                                         func=ACTF.Copy, scale=VSCALE, bias=VBIAS)

            # ---- layer 1 spike scan ----
            # Depth-2 form: W_t = vhat_t - rho*A_{t-1}; s_t = (B_t <= W_t);
            # A_t = rho*A_{t-1} + B_t; B_{t+1} = rho*B_t + s_t.
            # Per-step critical chain is only 2 ops (s_t<-W_t, B<-s_t); the
            # other two ops' inputs are >=2 ops old, hiding the SBUF
            # read-after-write bubble that dominates small VectorE ops.
            w_tmp = scan.tile([128, G], f32)
            for t in range(T):
                nc.vector.scalar_tensor_tensor(w_tmp, a_st, -RHO, v_all[:, t, :],
                                               OP.mult, OP.add)
                nc.vector.scalar_tensor_tensor(a_st, a_st, RHO, b_st, OP.mult, OP.add)
                nc.vector.tensor_tensor(s_all[:, t + 1, :], b_st, w_tmp, OP.is_le)
                nc.vector.scalar_tensor_tensor(b_st, b_st, RHO, s_all[:, t + 1, :],
                                               OP.mult, OP.add)

            # ---- layer 2: z2T[t, (b,o)] = s1^T W2^T ; a2 = K^T-conv ----
            z2t_sb = l2p.tile([128, 3, B_LOC * NOUT], bfl)
            for tc_i, (toff, tsz) in enumerate(TCH):
                z2ps = mmps.tile([128, B_LOC * NOUT], f32, tag="mmps")
                for b in range(B_LOC):
                    for mt in range(MT_N):
                        g = b * MT_N + mt
                        nc.tensor.matmul(
                            z2ps[:tsz, b * NOUT:(b + 1) * NOUT],
                            s_all[:, 1 + toff:1 + toff + tsz, g],
                            w2t_sb[:, mt, :],
                            start=(mt == 0), stop=(mt == MT_N - 1),
                        )
                nc.scalar.activation(out=z2t_sb[:tsz, tc_i, :], in_=z2ps[:tsz, :],
                                     func=ACTF.Copy)

            a2ps = mmps.tile([B_LOC * NOUT, T], f32, tag="mmps")
            for cj, (tj, szj) in enumerate(TCH):
                cis = [cj] if cj == 0 else [cj - 1, cj]
                for idx, ci in enumerate(cis):
                    ti, szi = TCH[ci]
                    nc.tensor.matmul(
                        a2ps[:, tj:tj + szj],
                        z2t_sb[:szi, ci, :],
                        kmat_sb[:szi, ci, tj:tj + szj],
                        start=(idx == 0), stop=(idx == len(cis) - 1),
                    )
            v2 = l2p.tile([B_LOC * NOUT, T], f32)
            nc.scalar.activation(out=v2, in_=a2ps, func=ACTF.Copy,
                                 scale=VSCALE, bias=VBIAS)

            # ---- layer 2 spike scan via fixpoint (never near threshold) ----
            s2 = l2p.tile([B_LOC * NOUT, T + 2], bfl)
            nc.vector.memset(s2[:, 0:1], 0.0)
            nc.vector.tensor_scalar(s2[:, 1:T + 1], v2, 0.0, None, OP.is_ge)
            out_sb = l2p.tile([B_LOC * NOUT, T], f32)
            P = B_LOC * NOUT
            for it in range(K2_FIX - 1):
                x1 = l2p.tile([P, T], f32, tag="x1")
                x2 = l2p.tile([P, T], f32, tag="x2")
                nc.vector.tensor_tensor_scan(x1, rho_sb[:P, :], s2[:, 0:T], 0.0,
                                             OP.mult, OP.add)
                nc.vector.tensor_tensor_scan(x2, rho_sb[:P, :], x1, 0.0,
                                             OP.mult, OP.add)
                last = it == K2_FIX - 2
                nc.vector.tensor_tensor(out_sb if last else s2[:, 1:T + 1],
                                        x2, v2, OP.is_le)

            nc.sync.dma_start(out=out_d.rearrange("b o t -> (b o) t"), in_=out_sb)
            if debug_taps:
                nc.sync.dma_start(out=dbg_v, in_=v_all)
                nc.sync.dma_start(out=dbg_s, in_=s_all[:, 1:T + 1, :])
                nc.sync.dma_start(out=dbg_v2, in_=v2)

    nc.compile()
    return nc


def _to_bf16_binary(x):
    # spike values are exactly 0.0/1.0, which bf16 represents exactly
    return x.astype(bf16)


def kernel(spike_input: np.ndarray, W1: np.ndarray, W2: np.ndarray) -> np.ndarray:
    from concourse.bass_utils import run_bass_kernel_spmd

    nc = build_program()

    xb = _to_bf16_binary(np.ascontiguousarray(spike_input, dtype=np.float32))
    w1t = np.ascontiguousarray(W1.T).astype(bf16)
    w2t = np.ascontiguousarray(W2.T).astype(bf16)

    in_maps = []
    for c in range(N_CORES):
        in_maps.append({
            "x": np.ascontiguousarray(xb[c * B_LOC:(c + 1) * B_LOC]),
            "w1t": w1t,
            "w2t": w2t,
        })
    res = run_bass_kernel_spmd(nc, in_maps, core_ids=list(range(N_CORES)))
    out = np.concatenate([r["out"] for r in res.results], axis=0)
    return np.ascontiguousarray(out, dtype=np.float32)


def _prep_in_maps(spike_input, W1, W2):
    xb = _to_bf16_binary(np.ascontiguousarray(spike_input, dtype=np.float32))
    w1t = np.ascontiguousarray(W1.T).astype(bf16)
    w2t = np.ascontiguousarray(W2.T).astype(bf16)
    return [
        {"x": np.ascontiguousarray(xb[c * B_LOC:(c + 1) * B_LOC]),
         "w1t": w1t, "w2t": w2t}
        for c in range(N_CORES)
    ]


def _ensure_ntff_hook():
    """The RL container's antenv stub lacks axon_hooks; synthesize it and
    register the ctypes NTFF profiler from trn_agent_boot."""
    import sys
    import types
    try:
        from antenv.axon_hooks import get_axon_ntff_profile_hook  # noqa: F401
        return
    except ImportError:
        pass
    import antenv
    mod = types.ModuleType("antenv.axon_hooks")
    store = {"h": None}
    mod.set_axon_ntff_profile_hook = lambda h: store.__setitem__("h", h)
    mod.get_axon_ntff_profile_hook = lambda: store["h"]
    sys.modules["antenv.axon_hooks"] = mod
    antenv.axon_hooks = mod
    from trn_agent_boot.trn_boot import _ntff_profile_via_ctypes
    mod.set_axon_ntff_profile_hook(_ntff_profile_via_ctypes("/opt/axon/libaxon_pjrt.so"))


def profile_hw(inputs):
    """Run with NTFF tracing; return max-core exec time in ns (or None)."""
    from concourse.bass_utils import run_bass_kernel_spmd

    _ensure_ntff_hook()
    nc = build_program()
    in_maps = _prep_in_maps(**inputs)
    res = run_bass_kernel_spmd(nc, in_maps, core_ids=list(range(N_CORES)),
                               trace=True)
    return res.exec_time_ns


if __name__ == "__main__":
    x = np.zeros((B_FULL, NIN, T), np.float32)
    w1 = np.zeros((NHID, NIN), np.float32)
    w2 = np.zeros((NOUT, NHID), np.float32)
    print(kernel(x, w1, w2).shape)


# revision 40
# speedup vs baseline: 1.0956x; 1.0551x over previous
"""SLAYER SNN forward kernel for Trainium2 (8 NeuronCores, data-parallel over batch).

Network (per reference): x:[B,2048,350] -> psp(srm) -> W1 -> spike-scan ->
psp(srm) -> W2 -> spike-scan -> s2:[B,10,350].

Math restructuring (exact up to fp32 reassociation):
  - psp is a causal linear filter along t; it commutes with the dense layer:
      a1 = einsum(psp(x), W1) == psp(einsum(x, W1))
    so the big matmul runs on the raw binary spikes (exact in bf16) and the
    100-tap filter runs on the small [512, T] result as a banded-Toeplitz
    matmul against a constant K matrix.
  - the refractory kernel refk[j] = -20 * j * e^(1-j) is the impulse response
    of a 2nd-order linear recurrence (double pole at rho=e^-1).  With scaled
    states P,Q (P = refractory potential / (-20), dividing by -20 flips the
    threshold comparison):
        s[t] = (P[t] <= vhat[t]),   vhat = (u - theta)/20
        Q <- rho*Q + s[t]
        P <- rho*P + Q
    The scan is emitted as 4 fused VectorE ops per step in a depth-2 schedule
    (W = vhat - rho*P precomputed) so only two ops per step sit on the
    semaphore-latency-bound dependency chain; all 2048 neuron-batch units per
    core advance together ([128, 16] per op, t-major buffers so per-step
    slices are contiguous).
  - layer 2 never comes near threshold (|a2| < 1 << theta=10), so its scan is
    computed by fixpoint iteration: bulk IIR scans along t (tensor_tensor_scan)
    + threshold, iterated K2=3 times — exact unless a 3-long chained
    refractory flip-cascade exists, impossible with a 9+ margin to theta.

Sharding: batch 32 -> 8 cores x 4.  W1/W2/K replicated.
"""

import numpy as np
import ml_dtypes

B_FULL = 32
N_CORES = 8
B_LOC = B_FULL // N_CORES  # 4
NIN = 2048
NHID = 512
NOUT = 10
T = 350
THETA = 10.0
K_SRM = 100

NC_IN = NIN // 128  # 16 contraction chunks
MT_N = NHID // 128  # 4 hidden m-tiles
G = B_LOC * MT_N    # 16 scan groups per core
TCH = [(0, 128), (128, 128), (256, 94)]  # (offset, size) t' chunks
RHO = float(np.float32(np.exp(np.float64(-1.0))))
CRHO = -20.0  # refk[1]; refk[j] = CRHO * j * RHO**(j-1)
VSCALE = 0.05         # 1/|CRHO|, exact in fp32
VBIAS = -0.5          # -THETA/|CRHO|, exact
# spike condition: m = u + CRHO*P >= theta  <=>  P <= (u-theta)/20 = vhat
K2_FIX = 3

bf16 = ml_dtypes.bfloat16


def _srm_np():
    t = np.arange(K_SRM, dtype=np.float32)
    return ((t / np.float32(10.0)) * np.exp(np.float32(1.0) - t / np.float32(10.0))).astype(np.float32)


def _kmat_np():
    """K[c, p, t] = srm[t - (128c + p)], zero outside [0, K_SRM)."""
    srm = _srm_np()
    k = np.zeros((3, 128, T), dtype=np.float32)
    for c in range(3):
        for p in range(TCH[c][1]):
            tp = 128 * c + p
            j0, j1 = tp, min(T, tp + K_SRM)
            k[c, p, j0:j1] = srm[: j1 - j0]
    return k


def build_program(debug_taps: bool = False):
    import concourse.bass as bass
    import concourse.tile as tile
    from concourse import bacc, mybir

    f32 = mybir.dt.float32
    bfl = mybir.dt.bfloat16
    OP = mybir.AluOpType
    ACTF = mybir.ActivationFunctionType

    nc = bacc.Bacc("TRN2", target_bir_lowering=False, debug=False,
                   enable_asserts=False, num_devices=N_CORES)

    x_d = nc.dram_tensor("x", [B_LOC, NIN, T], bfl, kind="ExternalInput").ap()
    w1t_d = nc.dram_tensor("w1t", [NIN, NHID], bfl, kind="ExternalInput").ap()
    w2t_d = nc.dram_tensor("w2t", [NHID, NOUT], bfl, kind="ExternalInput").ap()
    out_d = nc.dram_tensor("out", [B_LOC, NOUT, T], f32, kind="ExternalOutput").ap()
    kmat_d = nc.inline_tensor(_kmat_np().astype(bf16), name="kmat").ap()
    if debug_taps:
        dbg_v = nc.dram_tensor("dbg_v", [128, T, G], f32, kind="ExternalOutput").ap()
        dbg_s = nc.dram_tensor("dbg_s", [128, T, G], mybir.dt.bfloat16,
                               kind="ExternalOutput").ap()
        dbg_v2 = nc.dram_tensor("dbg_v2", [B_LOC * NOUT, T], f32,
                                kind="ExternalOutput").ap()

    with tile.TileContext(nc) as tc:
        with (
            tc.tile_pool(name="singles", bufs=1) as singles,
            tc.tile_pool(name="xin", bufs=1) as xin,
            tc.tile_pool(name="z1sb", bufs=1) as z1sb,
            tc.tile_pool(name="scan", bufs=1) as scan,
            tc.tile_pool(name="l2", bufs=1) as l2p,
            tc.tile_pool(name="zps", bufs=4, space="PSUM") as zps,
            tc.tile_pool(name="mmps", bufs=3, space="PSUM") as mmps,
        ):
            # ---- constants ----
            w1t_sb = singles.tile([128, NC_IN, NHID], bfl)
            for c4 in range(4):
                nc.sync.dma_start(
                    out=w1t_sb[:, c4 * 4:(c4 + 1) * 4, :],
                    in_=w1t_d[c4 * 512:(c4 + 1) * 512].rearrange(
                        "(c p) m -> p c m", p=128))
            w2t_sb = singles.tile([128, MT_N, NOUT], bfl)
            nc.gpsimd.dma_start(out=w2t_sb, in_=w2t_d.rearrange("(c p) o -> p c o", p=128))
            kmat_sb = singles.tile([128, 3, T], bfl)
            for c in range(3):
                nc.gpsimd.dma_start(out=kmat_sb[:, c, :], in_=kmat_d[c])
            rho_sb = singles.tile([128, T], f32)
            nc.vector.memset(rho_sb, RHO)

            # ---- persistent scan buffers (t-major: per-step slices contiguous) ----
            v_all = scan.tile([128, T, G], f32)       # vhat for all 16 groups
            s_all = scan.tile([128, T + 2, G], bfl)   # t=0 slice = zero guard
            a_st = scan.tile([128, G], f32)           # P state
            b_st = scan.tile([128, G], f32)           # Q state
            nc.vector.memset(s_all[:, 0, :], 0.0)
            nc.vector.memset(a_st, 0.0)
            nc.vector.memset(b_st, 0.0)

            # ---- layer 1, t-chunk-major so the scan can start after chunk 0:
            # for each t'-chunk: z1T chunk for all batches, then a1/vhat for
            # the t-columns this chunk completes.
            x_tiles = []
            dma_engines = [nc.gpsimd, nc.sync, nc.gpsimd, nc.sync]
            for b in range(B_LOC):
                x_sb = xin.tile([128, NC_IN, T], bfl, tag=f"x{b}", name=f"x_sb{b}")
                for c4 in range(4):
                    dma_engines[b].dma_start(
                        out=x_sb[:, c4 * 4:(c4 + 1) * 4, :],
                        in_=x_d[b][c4 * 512:(c4 + 1) * 512].rearrange(
                            "(c p) t -> p c t", p=128))
                x_tiles.append(x_sb)
            z1_tiles = [z1sb.tile([128, 3, NHID], bfl, tag=f"z1{b}", name=f"z1t{b}")
                        for b in range(B_LOC)]

            # Two phases so the scan can start ~18us earlier: phase A covers
            # t'-chunks 0,1 (completing vhat cols 0..255 for every group);
            # phase B (chunk 2, cols 256..349) overlaps the scan's first steps.
            def stage_b(b, tc_i, toff, tsz):
                z1ps = zps.tile([128, NHID], f32, tag="zps", name=f"z1ps{b}_{tc_i}")
                for ncnk in range(NC_IN):
                    nc.tensor.matmul(
                        z1ps[:tsz, :],
                        x_tiles[b][:, ncnk, toff:toff + tsz],
                        w1t_sb[:, ncnk, :],
                        start=(ncnk == 0), stop=(ncnk == NC_IN - 1),
                    )
                nc.scalar.activation(out=z1_tiles[b][:tsz, tc_i, :],
                                     in_=z1ps[:tsz, :], func=ACTF.Copy)

            for b in range(B_LOC):
                stage_b(b, 0, *TCH[0])
            for b in range(B_LOC):
                for mt in range(MT_N):
                    g = b * MT_N + mt
                    a1ps = mmps.tile([128, 128], f32, tag="mmps", name=f"a1psA0{g}")
                    nc.tensor.matmul(a1ps[:, :],
                                     z1_tiles[b][:128, 0, mt * 128:(mt + 1) * 128],
                                     kmat_sb[:128, 0, 0:128],
                                     start=True, stop=True)
                    nc.scalar.activation(out=v_all[:, 0:128, g], in_=a1ps,
                                         func=ACTF.Copy, scale=VSCALE, bias=VBIAS)
            for b in range(B_LOC):
                stage_b(b, 1, *TCH[1])
            for b in range(B_LOC):
                for mt in range(MT_N):
                    g = b * MT_N + mt
                    a1ps = mmps.tile([128, 128], f32, tag="mmps", name=f"a1psA1{g}")
                    nc.tensor.matmul(a1ps[:, :],
                                     z1_tiles[b][:128, 0, mt * 128:(mt + 1) * 128],
                                     kmat_sb[:128, 0, 128:256],
                                     start=True, stop=False)
                    nc.tensor.matmul(a1ps[:, :],
                                     z1_tiles[b][:128, 1, mt * 128:(mt + 1) * 128],
                                     kmat_sb[:128, 1, 128:256],
                                     start=False, stop=True)
                    nc.scalar.activation(out=v_all[:, 128:256, g], in_=a1ps,
                                         func=ACTF.Copy, scale=VSCALE, bias=VBIAS)
            for b in range(B_LOC):
                stage_b(b, 2, *TCH[2])
            for b in range(B_LOC):
                for mt in range(MT_N):
                    g = b * MT_N + mt
                    a1ps = mmps.tile([128, 94], f32, tag="mmps", name=f"a1psB{g}")
                    nc.tensor.matmul(a1ps[:, :],
                                     z1_tiles[b][:128, 1, mt * 128:(mt + 1) * 128],
                                     kmat_sb[:128, 1, 256:350],
                                     start=True, stop=False)
                    nc.tensor.matmul(a1ps[:, :],
                                     z1_tiles[b][:94, 2, mt * 128:(mt + 1) * 128],
                                     kmat_sb[:94, 2, 256:350],
                                     start=False, stop=True)
                    nc.scalar.activation(out=v_all[:, 256:350, g], in_=a1ps,
                                         func=ACTF.Copy, scale=VSCALE, bias=VBIAS)

            # ---- layer 1 spike scan ----
            # Depth-2 form: W_t = vhat_t - rho*A_{t-1}; s_t = (B_t <= W_t);
            # A_t = rho*A_{t-1} + B_t; B_{t+1} = rho*B_t + s_t.
            # Per-step critical chain is only 2 ops (s_t<-W_t, B<-s_t); the
            # other two ops' inputs are >=2 ops old, hiding the SBUF
            # read-after-write bubble that dominates small VectorE ops.
            w_tmp = scan.tile([128, G], f32)
            for t in range(T):
                nc.vector.scalar_tensor_tensor(w_tmp, a_st, -RHO, v_all[:, t, :],
                                               OP.mult, OP.add)
                nc.vector.scalar_tensor_tensor(a_st, a_st, RHO, b_st, OP.mult, OP.add)
                nc.vector.tensor_tensor(s_all[:, t + 1, :], b_st, w_tmp, OP.is_le)
                nc.vector.scalar_tensor_tensor(b_st, b_st, RHO, s_all[:, t + 1, :],
                                               OP.mult, OP.add)

            # ---- layer 2: z2T[t, (b,o)] = s1^T W2^T ; a2 = K^T-conv ----
            z2t_sb = l2p.tile([128, 3, B_LOC * NOUT], bfl)
            for tc_i, (toff, tsz) in enumerate(TCH):
                z2ps = mmps.tile([128, B_LOC * NOUT], f32, tag="mmps")
                for b in range(B_LOC):
                    for mt in range(MT_N):
                        g = b * MT_N + mt
                        nc.tensor.matmul(
                            z2ps[:tsz, b * NOUT:(b + 1) * NOUT],
                            s_all[:, 1 + toff:1 + toff + tsz, g],
                            w2t_sb[:, mt, :],
                            start=(mt == 0), stop=(mt == MT_N - 1),
                        )
                nc.scalar.activation(out=z2t_sb[:tsz, tc_i, :], in_=z2ps[:tsz, :],
                                     func=ACTF.Copy)

            a2ps = mmps.tile([B_LOC * NOUT, T], f32, tag="mmps")
            for cj, (tj, szj) in enumerate(TCH):
                cis = [cj] if cj == 0 else [cj - 1, cj]
                for idx, ci in enumerate(cis):
                    ti, szi = TCH[ci]
                    nc.tensor.matmul(
                        a2ps[:, tj:tj + szj],
                        z2t_sb[:szi, ci, :],
                        kmat_sb[:szi, ci, tj:tj + szj],
                        start=(idx == 0), stop=(idx == len(cis) - 1),
                    )
            v2 = l2p.tile([B_LOC * NOUT, T], f32)
            nc.scalar.activation(out=v2, in_=a2ps, func=ACTF.Copy,
                                 scale=VSCALE, bias=VBIAS)

            # ---- layer 2 spike scan via fixpoint (never near threshold) ----
            s2 = l2p.tile([B_LOC * NOUT, T + 2], bfl)
            nc.vector.memset(s2[:, 0:1], 0.0)
            nc.vector.tensor_scalar(s2[:, 1:T + 1], v2, 0.0, None, OP.is_ge)
            out_sb = l2p.tile([B_LOC * NOUT, T], f32)
            P = B_LOC * NOUT
            for it in range(K2_FIX - 1):
                x1 = l2p.tile([P, T], f32, tag="x1")
                x2 = l2p.tile([P, T], f32, tag="x2")
                nc.vector.tensor_tensor_scan(x1, rho_sb[:P, :], s2[:, 0:T], 0.0,
                                             OP.mult, OP.add)
                nc.vector.tensor_tensor_scan(x2, rho_sb[:P, :], x1, 0.0,
                                             OP.mult, OP.add)
                last = it == K2_FIX - 2
                nc.vector.tensor_tensor(out_sb if last else s2[:, 1:T + 1],
                                        x2, v2, OP.is_le)

            nc.sync.dma_start(out=out_d.rearrange("b o t -> (b o) t"), in_=out_sb)
            if debug_taps:
                nc.sync.dma_start(out=dbg_v, in_=v_all)
                nc.sync.dma_start(out=dbg_s, in_=s_all[:, 1:T + 1, :])
                nc.sync.dma_start(out=dbg_v2, in_=v2)

    nc.compile()
    return nc


def _to_bf16_binary(x):
    # spike values are exactly 0.0/1.0, which bf16 represents exactly
    return x.astype(bf16)


def kernel(spike_input: np.ndarray, W1: np.ndarray, W2: np.ndarray) -> np.ndarray:
    from concourse.bass_utils import run_bass_kernel_spmd

    nc = build_program()

    xb = _to_bf16_binary(np.ascontiguousarray(spike_input, dtype=np.float32))
    w1t = np.ascontiguousarray(W1.T).astype(bf16)
    w2t = np.ascontiguousarray(W2.T).astype(bf16)

    in_maps = []
    for c in range(N_CORES):
        in_maps.append({
            "x": np.ascontiguousarray(xb[c * B_LOC:(c + 1) * B_LOC]),
            "w1t": w1t,
            "w2t": w2t,
        })
    res = run_bass_kernel_spmd(nc, in_maps, core_ids=list(range(N_CORES)))
    out = np.concatenate([r["out"] for r in res.results], axis=0)
    return np.ascontiguousarray(out, dtype=np.float32)


def _prep_in_maps(spike_input, W1, W2):
    xb = _to_bf16_binary(np.ascontiguousarray(spike_input, dtype=np.float32))
    w1t = np.ascontiguousarray(W1.T).astype(bf16)
    w2t = np.ascontiguousarray(W2.T).astype(bf16)
    return [
        {"x": np.ascontiguousarray(xb[c * B_LOC:(c + 1) * B_LOC]),
         "w1t": w1t, "w2t": w2t}
        for c in range(N_CORES)
    ]


def _ensure_ntff_hook():
    """The RL container's antenv stub lacks axon_hooks; synthesize it and
    register the ctypes NTFF profiler from trn_agent_boot."""
    import sys
    import types
    try:
        from antenv.axon_hooks import get_axon_ntff_profile_hook  # noqa: F401
        return
    except ImportError:
        pass
    import antenv
    mod = types.ModuleType("antenv.axon_hooks")
    store = {"h": None}
    mod.set_axon_ntff_profile_hook = lambda h: store.__setitem__("h", h)
    mod.get_axon_ntff_profile_hook = lambda: store["h"]
    sys.modules["antenv.axon_hooks"] = mod
    antenv.axon_hooks = mod
    from trn_agent_boot.trn_boot import _ntff_profile_via_ctypes
    mod.set_axon_ntff_profile_hook(_ntff_profile_via_ctypes("/opt/axon/libaxon_pjrt.so"))


def profile_hw(inputs):
    """Run with NTFF tracing; return max-core exec time in ns (or None)."""
    from concourse.bass_utils import run_bass_kernel_spmd

    _ensure_ntff_hook()
    nc = build_program()
    in_maps = _prep_in_maps(**inputs)
    res = run_bass_kernel_spmd(nc, in_maps, core_ids=list(range(N_CORES)),
                               trace=True)
    return res.exec_time_ns


if __name__ == "__main__":
    x = np.zeros((B_FULL, NIN, T), np.float32)
    w1 = np.zeros((NHID, NIN), np.float32)
    w2 = np.zeros((NOUT, NHID), np.float32)
    print(kernel(x, w1, w2).shape)
